# revision 1
# baseline (speedup 1.0000x reference)
"""ConvBlock (fake-quant conv3x3 + sync-BN + ReLU6) on 8 Trainium2 NeuronCores.

Strategy (data-parallel, batch 32 -> 4 images/core):
- Per-core partition layout p = half*64 + img*16 + ci: each partition holds one
  (image-half, channel) spatial block of 114x226 zero-padded rows/cols.
- Fake-quant is exact-integer: xq_int=round(x*127/amax) stored bf16 (|v|<=127,
  exact), conv = 9 accumulating 128x128 bf16 matmuls per 2-row chunk with
  block-diagonal weights (8 blocks x 16ch); all shifts are AP offsets.
  PSUM accumulates exact fp32 integers (<2^24). Scales fold into BN affine.
- Global amax via AllReduce(max), sync-BN sums via AllReduce(add).
"""
import os
import time
import numpy as np
import ml_dtypes

import concourse.bacc as bacc
import concourse.bass_isa as bass_isa
import concourse.mybir as mybir
import concourse.tile as tile
from concourse import bass_utils

N_CORES = 8
IMGS = 4            # images per core
CH = 16
H = W = 224
HH = 112            # half-image rows
RS = 114            # stored rows per block (halo + pads)
CS = 226            # stored cols (224 + 2 pad)
GR = 2              # guard rows each side of xq
FLAT = RS * CS      # 25764
NBAND = 8
BROWS = 14          # output rows per band (112 = 8*14)
NCHUNK = 56         # 2-output-row conv chunks
GSZ = 6             # conv chunks in flight (= PSUM banks for conv)
MAGIC = 12582912.0  # 1.5 * 2^23: (x + MAGIC) - MAGIC == round-to-nearest-even
QP = 127.0
M_GLOBAL = float(32 * 224 * 224)
BN_EPS = 1e-5

f32 = mybir.dt.float32
bf16 = mybir.dt.bfloat16

KPHASE = int(os.environ.get("KPHASE", "5"))
KSUB = int(os.environ.get("KSUB", "9"))
_CACHE = {}


def _build_nc():
    nc = bacc.Bacc("TRN2", target_bir_lowering=False, debug=False,
                   num_devices=N_CORES)
    x_d = nc.dram_tensor("x", [IMGS, CH, H, W], f32, kind="ExternalInput")
    wq_d = nc.dram_tensor("wq", [9, 128, 128], bf16, kind="ExternalInput")
    e_d = nc.dram_tensor("e_mat", [128, CH], f32, kind="ExternalInput")
    e2_d = nc.dram_tensor("e2_mat", [CH, 128], f32, kind="ExternalInput")
    gam_d = nc.dram_tensor("gamma_p", [128, 1], f32, kind="ExternalInput")
    bet_d = nc.dram_tensor("beta_p", [128, 1], f32, kind="ExternalInput")
    wsc_d = nc.dram_tensor("wsc", [128, 1], f32, kind="ExternalInput")
    y_d = nc.dram_tensor("y", [IMGS, CH, H, W], f32, kind="ExternalOutput")

    AF = mybir.ActivationFunctionType
    ALU = mybir.AluOpType
    RG = [list(range(N_CORES))]
    with tile.TileContext(nc) as tc:
        with (
            tc.tile_pool(name="persist", bufs=1) as sb,
            tc.tile_pool(name="ps", bufs=1, space="PSUM") as ps,
            tc.tile_pool(name="dram", bufs=1, space="DRAM") as dram,
        ):
            # ---- constants / weights ----
            lhsT = sb.tile([128, 9, 128], bf16)
            nc.sync.dma_start(lhsT[:], wq_d[:].rearrange("t p m -> p t m"))
            e_sb = sb.tile([128, CH], f32)
            nc.sync.dma_start(e_sb[:], e_d[:])
            e2_sb = sb.tile([CH, 128], f32)
            nc.sync.dma_start(e2_sb[:], e2_d[:])
            gam_sb = sb.tile([128, 1], f32)
            nc.sync.dma_start(gam_sb[:], gam_d[:])
            bet_sb = sb.tile([128, 1], f32)
            nc.sync.dma_start(bet_sb[:], bet_d[:])
            wsc_sb = sb.tile([128, 1], f32)
            nc.sync.dma_start(wsc_sb[:], wsc_d[:])
            magic_sb = sb.tile([128, 1], f32)
            nc.vector.memset(magic_sb[:], MAGIC)
            am = sb.tile([128, 1], f32)
            alpha = sb.tile([128, 1], f32)
            s_inv = sb.tile([128, 1], f32)
            xq = sb.tile([128, RS + 2 * GR, CS], bf16)
            sums = sb.tile([128, NCHUNK], f32)
            sqs = sb.tile([128, NBAND], f32)

            with tc.tile_pool(name="px", bufs=1) as px:
                # ---- load x into padded per-block layout ----
                x_pad = px.tile([128, RS, CS], f32)
                nc.vector.memset(x_pad[:, 0, :], 0.0)
                nc.vector.memset(x_pad[:, RS - 1, :], 0.0)
                nc.vector.memset(x_pad[:, :, 0:1], 0.0)
                nc.vector.memset(x_pad[:, :, CS - 1:CS], 0.0)
                # half 0: input rows 0..112 -> stored rows 1..113
                nc.sync.dma_start(x_pad[0:64, 1:RS, 1:W + 1], x_d[:, :, 0:113, :])
                # half 1: input rows 111..223 -> stored rows 0..112
                nc.sync.dma_start(x_pad[64:128, 0:RS - 1, 1:W + 1],
                                  x_d[:, :, 111:224, :])
                x_flat = x_pad[:].rearrange("p a b -> p (a b)")

                if KPHASE >= 2:
                    # ---- global absmax -> quant scale ----
                    nc.vector.tensor_reduce(am[:], x_flat, mybir.AxisListType.X,
                                            ALU.max, apply_absolute_value=True)
                    ar1_in = dram.tile([128, 1], f32)
                    ar1_out = dram.tile([128, 1], f32)
                    nc.sync.dma_start(ar1_in[:], am[:])
                    nc.gpsimd.collective_compute(
                        "AllReduce", ALU.max, replica_groups=RG,
                        ins=[ar1_in[:].opt()], outs=[ar1_out[:].opt()])
                    am_g = px.tile([128, 1], f32)
                    nc.sync.dma_start(am_g[:], ar1_out[:])
                    nc.gpsimd.partition_all_reduce(alpha[:], am_g[:], 128,
                                                   bass_isa.ReduceOp.max)
                    rcp_a = px.tile([128, 1], f32)
                    nc.vector.reciprocal(rcp_a[:], alpha[:])
                    nc.vector.tensor_scalar_mul(s_inv[:], rcp_a[:], QP)

                    # ---- quantize to integer-valued bf16 (banded) ----
                    nc.vector.memset(xq[:, 0:GR, :], 0.0)
                    nc.vector.memset(xq[:, GR + RS:, :], 0.0)
                    xq_flat = xq[:, GR:GR + RS, :].rearrange("p a b -> p (a b)")
                    qb = [round(i * FLAT / NBAND) for i in range(NBAND + 1)]
                    qmax = max(qb[i + 1] - qb[i] for i in range(NBAND))
                    for i in range(NBAND):
                        lo, hi = qb[i], qb[i + 1]
                        qtmp = px.tile([128, qmax], f32, tag="qtmp", bufs=2)
                        nc.scalar.activation(qtmp[:, 0:hi - lo], x_flat[:, lo:hi],
                                             AF.Identity, bias=magic_sb[:],
                                             scale=s_inv[:])
                        nc.vector.tensor_scalar_add(xq_flat[:, lo:hi],
                                                    qtmp[:, 0:hi - lo], -MAGIC)

            if KPHASE >= 3:
                with tc.tile_pool(name="py", bufs=1) as py:
                    # ---- conv: 56 chunks x 9 taps, block-diag matmuls ----
                    ybands = [py.tile([128, BROWS, W], f32, tag=f"yb{b}",
                                      name=f"yband{b}") for b in range(NBAND)]
                    ngroups = (NCHUNK + GSZ - 1) // GSZ
                    for g in range(ngroups):
                        chunks = list(range(GSZ * g, min(GSZ * g + GSZ, NCHUNK)))
                        pts = {}
                        for t in range(9):
                            kh, kw = t // 3, t % 3
                            for c in chunks:
                                if t == 0:
                                    pts[c] = ps.tile([128, 2, W], f32,
                                                     tag="cvp", bufs=GSZ,
                                                     name="cvp")
                                r0 = GR + 2 * c + kh
                                nc.tensor.matmul(pts[c][:], lhsT[:, t, :],
                                                 xq[:, r0:r0 + 2, kw:kw + W],
                                                 start=(t == 0), stop=(t == 8))
                        for c in chunks:
                            b, k = c // 7, c % 7
                            nc.vector.tensor_scalar(
                                ybands[b][:, 2 * k:2 * k + 2, :], pts[c][:],
                                0.0, 0.0, ALU.add, ALU.add,
                                accum_out=sums[:, c:c + 1])

                    if KPHASE >= 4:
                        # ---- per-band sum of squares ----
                        for b in range(NBAND):
                            sq_scr = py.tile([128, BROWS, W], f32, tag="scr",
                                             bufs=4)
                            nc.scalar.activation(sq_scr[:], ybands[b][:],
                                                 AF.Square)
                            nc.vector.tensor_reduce(
                                sqs[:, b:b + 1],
                                sq_scr[:].rearrange("p a b -> p (a b)"),
                                mybir.AxisListType.X, ALU.add)

                        # ---- sync-BN: all-reduce per-channel sums ----
                        if True:
                            s1 = sb.tile([128, 1], f32)
                            nc.vector.tensor_reduce(s1[:], sums[:],
                                                    mybir.AxisListType.X, ALU.add)
                            s2 = sb.tile([128, 1], f32)
                            nc.vector.tensor_reduce(s2[:], sqs[:],
                                                    mybir.AxisListType.X, ALU.add)
                            st2 = sb.tile([128, 2], f32)
                            nc.vector.tensor_copy(st2[:, 0:1], s1[:])
                            nc.vector.tensor_copy(st2[:, 1:2], s2[:])
                            pch = ps.tile([CH, 2], f32, tag="pstat", bufs=1,
                                          name="pch")
                            nc.tensor.matmul(pch[:], e_sb[:], st2[:], start=True,
                                             stop=True)
                            ch_sb = sb.tile([CH, 2], f32)
                            nc.vector.tensor_copy(ch_sb[:], pch[:])
                            ar2_in = dram.tile([CH, 2], f32)
                            ar2_out = dram.tile([CH, 2], f32)
                            nc.sync.dma_start(ar2_in[:], ch_sb[:])
                            nc.gpsimd.collective_compute(
                                "AllReduce", ALU.add, replica_groups=RG,
                                ins=[ar2_in[:].opt()], outs=[ar2_out[:].opt()])
                            g16 = sb.tile([CH, 2], f32)
                            nc.sync.dma_start(g16[:], ar2_out[:])
                            pbc = ps.tile([128, 2], f32, tag="pbc", bufs=1,
                                          name="pbc")
                            nc.tensor.matmul(pbc[:], e2_sb[:], g16[:], start=True,
                                             stop=True)

                            # ---- BN affine coefficients ----
                            TT = nc.vector.tensor_tensor
                            mean_i = sb.tile([128, 1], f32)
                            nc.vector.tensor_scalar(mean_i[:], pbc[:, 0:1],
                                                    1.0 / M_GLOBAL, None, ALU.mult)
                            ex2 = sb.tile([128, 1], f32)
                            nc.vector.tensor_scalar(ex2[:], pbc[:, 1:2],
                                                    1.0 / M_GLOBAL, None, ALU.mult)
                            msq = sb.tile([128, 1], f32)
                            TT(msq[:], mean_i[:], mean_i[:], ALU.mult)
                            var_i = sb.tile([128, 1], f32)
                            TT(var_i[:], ex2[:], msq[:], ALU.subtract)
                            s_phys = sb.tile([128, 1], f32)
                            TT(s_phys[:], alpha[:], wsc_sb[:], ALU.mult)
                            mean_p = sb.tile([128, 1], f32)
                            TT(mean_p[:], mean_i[:], s_phys[:], ALU.mult)
                            var_p = sb.tile([128, 1], f32)
                            nc.vector.tensor_scalar(var_p[:], var_i[:], s_phys[:],
                                                    s_phys[:], ALU.mult, ALU.mult)
                            v_eps = sb.tile([128, 1], f32)
                            nc.vector.tensor_scalar_add(v_eps[:], var_p[:], BN_EPS)
                            sqv = sb.tile([128, 1], f32)
                            nc.scalar.activation(sqv[:], v_eps[:], AF.Sqrt)
                            r = sb.tile([128, 1], f32, name="rsq0")
                            nc.vector.reciprocal(r[:], sqv[:])
                            for it in range(2):  # Newton rsqrt refinement
                                t1 = sb.tile([128, 1], f32, tag="nw1", bufs=2,
                                             name="nw1")
                                TT(t1[:], v_eps[:], r[:], ALU.mult)
                                t2 = sb.tile([128, 1], f32, tag="nw2", bufs=2,
                                             name="nw2")
                                TT(t2[:], t1[:], r[:], ALU.mult)
                                t3 = sb.tile([128, 1], f32, tag="nw3", bufs=2,
                                             name="nw3")
                                nc.vector.tensor_scalar(t3[:], t2[:], -0.5, 1.5,
                                                        ALU.mult, ALU.add)
                                rn = sb.tile([128, 1], f32, tag="nw4", bufs=2,
                                             name="nw4")
                                TT(rn[:], r[:], t3[:], ALU.mult)
                                r = rn
                            inv = sb.tile([128, 1], f32)
                            TT(inv[:], gam_sb[:], r[:], ALU.mult)
                            a_p = sb.tile([128, 1], f32)
                            TT(a_p[:], inv[:], s_phys[:], ALU.mult)
                            mip = sb.tile([128, 1], f32)
                            TT(mip[:], mean_p[:], inv[:], ALU.mult)
                            b_p = sb.tile([128, 1], f32)
                            TT(b_p[:], bet_sb[:], mip[:], ALU.subtract)

                    if KPHASE >= 5:
                        # ---- apply BN + ReLU6, write out ----
                        for b in range(NBAND):
                            aptmp = py.tile([128, BROWS, W], f32, tag="scr",
                                            bufs=4, name="aptmp")
                            nc.scalar.activation(aptmp[:], ybands[b][:], AF.Relu,
                                                 bias=b_p[:], scale=a_p[:])
                            ob = py.tile([128, BROWS, W], f32, tag="scr",
                                         bufs=4, name="ob")
                            nc.vector.tensor_scalar_min(ob[:], aptmp[:], 6.0)
                            for h in range(2):
                                nc.sync.dma_start(
                                    y_d[:, :, h * HH + b * BROWS:
                                        h * HH + (b + 1) * BROWS, :],
                                    ob[h * 64:(h + 1) * 64, :, :])
    nc.compile()
    return nc


def _host_prep(weight, gamma, beta):
    """Quantize weights exactly like the reference; build block-diag lhsT."""
    w = np.asarray(weight, np.float32)
    alpha_w = np.abs(w).max()
    step_w = alpha_w / QP
    wq_int = np.clip(np.round(w / step_w), -QP, QP).astype(np.float32)
    lhsT = np.zeros((9, 128, 128), np.float32)
    for t in range(9):
        kh, kw = t // 3, t % 3
        blk = wq_int[:, :, kh, kw].T  # [ci, co]
        for b in range(8):
            lhsT[t, b * 16:b * 16 + 16, b * 16:b * 16 + 16] = blk
    e = np.zeros((128, CH), np.float32)
    e2 = np.zeros((CH, 128), np.float32)
    for p in range(128):
        e[p, p % CH] = 1.0
        e2[p % CH, p] = 1.0
    gam_p = np.asarray(gamma, np.float32)[np.arange(128) % CH].reshape(128, 1)
    bet_p = np.asarray(beta, np.float32)[np.arange(128) % CH].reshape(128, 1)
    wsc = np.full((128, 1), step_w / QP, np.float32)
    return {
        "wq": lhsT.astype(ml_dtypes.bfloat16),
        "e_mat": e, "e2_mat": e2,
        "gamma_p": gam_p, "beta_p": bet_p, "wsc": wsc,
    }


def kernel(x, weight, gamma, beta, _trace=False):
    if "nc" not in _CACHE:
        _CACHE["nc"] = _build_nc()
    nc = _CACHE["nc"]
    x = np.asarray(x, np.float32)
    shared = _host_prep(weight, gamma, beta)
    in_maps = []
    for i in range(N_CORES):
        m = dict(shared)
        m["x"] = np.ascontiguousarray(x[IMGS * i:IMGS * (i + 1)])
        in_maps.append(m)
    t0 = time.time()
    try:
        res = bass_utils.run_bass_kernel_spmd(nc, in_maps,
                                              core_ids=list(range(N_CORES)),
                                              trace=_trace)
    except ModuleNotFoundError:
        res = bass_utils.run_bass_kernel_spmd(nc, in_maps,
                                              core_ids=list(range(N_CORES)))
    kernel.last_exec_s = time.time() - t0
    out = np.concatenate([res.results[i]["y"] for i in range(N_CORES)], axis=0)
    kernel.last_results = res
    return out

